# revision 26
# baseline (speedup 1.0000x reference)
"""Trainium2 Bass kernel for ErosionP4 (P4 group-equivariant grayscale erosion).

Reference computation (shapes hardcoded):
  x: [B=4, G=4, H=96, W=96, C=4] fp32, kernel: [5, 5, 3, C=4, F=8] fp32
  out[b,g,h,w,f] = sum_c min_{k,dy,dx} ( ygp[b,g,k,h+dy,w+dx,c] - krev[g,dy,dx,k,c,f] )
  where ygp[b,g,k] = x[b, (g+k-1) mod 4] spatially padded with +inf and
  krev = the 4 planar rotations of the depth-rotated SE, spatially reversed.

Sharding: core -> (g = core//2, f-half = core%2).  Each core computes all 4
batches for one group-rotation g and 4 of the 8 filters.

Layout "cf-block": partition p = 32*c + 8*f + hs (c = channel, f = filter
within the half, hs = h//12), free dims (hb, b, w) with h = 12*hs + hb.
Every (c, f) pair owns its own partitions, so the per-tap SE value is a
per-partition scalar and ONE tensor_scalar sub + ONE tensor_tensor min of
[128, 4608] cover all four filters of a tap:

  per tap:  tmp = win - kk[(c,f)]   (ACT activation w/ bias, 1 elem/cy,
                                     or DVE tensor_scalar, 4 elem/cy)
            acc = min(acc, tmp)     (DVE tensor_tensor, 2 elem/cy)

The input plane data is replicated across the 4 f partition groups (host-side
prep; DMA is a few us, off the critical path).  The 75 taps are split between
ACT-sub and DVE-sub by a static greedy balance with HW-measured costs
(DVE min 2150ns, DVE sub 1075ns, ACT sub 3900ns per tap): DVE streams every
min (the 2-elem/cycle two-tensor-op wall) while ACT shoulders most subs.
This sits at the measured two-engine roofline (~190us steady-state) with
~6x fewer instructions and semaphores than a per-filter formulation, which
cuts the one-shot (startup + drain) cost of a single NEFF execution.

The channel sum happens on the host (c pieces live on different partitions).
The final tap is processed in 4 column chunks so each chunk's Pool-sequencer
SWDGE output DMA starts while the rest still computes.
"""

import os
from contextlib import ExitStack

import numpy as np

import concourse.bass as bass
import concourse.mybir as mybir
import concourse.tile as tile
from concourse.bass_utils import run_bass_kernel_spmd

B, G, H, W, C = 4, 4, 96, 96, 4
KH, KW, F = 5, 5, 8
PAD = 2
WP = W + PAD * 2  # 100
NTAP = 3 * KH * KW  # 75
N_CORES = 8
NP = 4  # batches per core
NF = F // 2  # filters per core
HS = 8  # h slices (partition sub-dim)
HB = 12  # h rows per slice (free dim)
HBP = HB + KH - 1  # 16 padded rows per slice
FREE = HB * NP * W  # 4608
BIG = 30000.0  # +inf stand-in that survives fp16

CFG_REPEAT = int(os.environ.get("KCFG_REPEAT", "1"))
# HW-measured per-tap costs (ns) for the static DVE/ACT schedule.
COST_DVE_SUB = float(os.environ.get("KCFG_DVE_SUB", "1100"))
COST_DVE_MIN = float(os.environ.get("KCFG_DVE_MIN", "2170"))
COST_ACT_SUB = float(os.environ.get("KCFG_ACT_COST", "3400"))

FP16 = mybir.dt.float16

_prog_cache = {}
LAST_RESULTS = None
_CORR = {}  # core -> [NF] channel-summed absorbed-tap offset (set by _make_in_map)


def _taps():
    """Tap order is free (min is commutative).  Entry 1 is swapped to an
    even-dx tap: its SE value is absorbed into the accumulator offset (the
    whole kk table is shifted by it and the host subtracts the channel-summed
    offset), so its min reads the input window directly — no sub and no
    ACT round-trip at the pipeline head.  Even dx keeps the window 4B-aligned
    for the min's 2x packed mode."""
    t = [(k, dy, dx) for k in range(3) for dy in range(KH) for dx in range(KW)]
    t[1], t[2] = t[2], t[1]  # t[1] = (0, 0, 2)
    return t


ABS_TI = 1  # index (in _taps() order) of the absorbed tap


def _schedule():
    """Greedy static assignment of the 75 tap-subs to 'dve' | 'act'.

    Simulates both engine timelines: a DVE-own tap costs DVE sub+min; an
    ACT-assist tap costs ACT the sub, and DVE pays the min once the sub has
    landed (cross-engine wait)."""
    t_dve = t_act = 0.0
    out = []
    for ti in range(NTAP):
        if ti == 0:
            out.append("dve")
            t_dve += COST_DVE_SUB  # init writes acc directly, no min
            continue
        end_dve = t_dve + COST_DVE_SUB + COST_DVE_MIN
        a_end = t_act + COST_ACT_SUB
        end_act = max(t_dve, a_end) + COST_DVE_MIN
        if end_dve <= end_act:
            out.append("dve")
            t_dve = end_dve
        else:
            out.append("act")
            t_act = a_end
            t_dve = end_act
    # the final tap feeds the chunked output mins; keep it off ACT so the
    # output DMAs never queue behind ACT's sub backlog
    out[-1] = "dve"
    # the greedy bifurcates between ~54 and ~72 ACT taps; ACT_EXTRA flips a
    # few more DVE taps to ACT (spread evenly) for finer balance control
    extra = int(os.environ.get("KCFG_ACT_EXTRA", "0"))
    if extra > 0:
        dve_idx = [i for i, e in enumerate(out) if e == "dve" and 0 < i < NTAP - 1]
        step = max(1, len(dve_idx) // extra)
        for j in dve_idx[::step][:extra]:
            out[j] = "act"
    return out


class _SplitDrainTC(tile.TileContext):
    """TileContext whose kernel-tail drain is split into one drain per sem
    lane: the stock single Drain carries a wait for every lane used, which
    overflows the CTRL struct's sync-wait encoding on this compiler."""

    def _drain_and_barrier(self, tick_clock, wait_clock):
        from concourse.tile_sem_assignment import N_PROCS
        from concourse.vector_clock import ScopedClock, VectorClock

        gc = tick_clock.global_clock
        ticks = [gc[p] for p in range(N_PROCS)]
        for p in range(N_PROCS):
            if ticks[p] <= 0:
                continue
            sub = [ticks[q] if q == p else 0 for q in range(N_PROCS)]
            d = self.nc.sync.drain()
            wait_clock.add_sem_waits(d.ins, ScopedClock({None: VectorClock(sub)}))

        self.nc.all_engine_barrier()
        assert self.sems is not None
        popped = self.nc._tile_sem_poison_stack.pop()
        assert popped is self._sem_poison
        self.nc.clear_and_free_semaphores(list(self.sems.allocated().values()))
        self.nc.all_engine_barrier()


def _strip_stale_same_engine_waits(nc, lag=7):
    """Drop same-engine sem waits whose producer finished >= `lag` own-engine
    instructions earlier.

    This tile version emits a sem wait for EVERY hazard, including same-engine
    WAW/WAR whose producers are long retired; compute ISA structs can encode
    only ONE sync wait, so a ring-buffer rewrite (same-engine WAW + cross-
    engine WAR) overflows codegen.  Engines issue in order and their writes
    land within a couple of instructions, so a same-engine wait on a producer
    `lag` instructions back is vacuous.  Recent same-engine waits (pipelined
    RAW guards) are kept.
    """
    strip_types = {
        "InstActivation", "InstTensorScalarPtr", "InstTensorTensor",
        "InstTensorScalar", "InstMemset", "InstCopy", "InstTensorCopy",
        "InstTensorReduce",
    }
    counts = {}
    fn = nc.m.functions[0]
    for blk in fn.blocks:
        for ins in blk.instructions:
            si = ins.sync_info
            if si is None:
                continue
            eng = getattr(ins, "engine", None)
            ename = getattr(eng, "name", None) or (str(eng).split(".")[-1] if eng else "")
            if si.on_wait and type(ins).__name__ in strip_types and ename in (
                "Activation", "DVE", "Pool", "PE"
            ):
                keep = []
                for w in si.on_wait:
                    nm = w.ant_name or ""
                    if (
                        nm.startswith(ename + "_")
                        and w.wait_mode == "sem-ge-imm"
                        and w.wait_value is not None
                        and counts.get(nm, 0) - w.wait_value >= lag
                    ):
                        continue
                    keep.append(w)
                if len(keep) != len(si.on_wait):
                    si.on_wait = keep
            for u in si.on_update or []:
                if u.ant_name:
                    counts[u.ant_name] = counts.get(u.ant_name, 0) + (u.update_value or 1)
    return nc


def _build_program(repeat=1):
    import concourse.tile_sem_assignment as _tsa

    _orig_swdge = _tsa.NUM_SWDGE_GLOBAL_SEMS
    _tsa.NUM_SWDGE_GLOBAL_SEMS = 4
    try:
        return _strip_stale_same_engine_waits(_build_program_inner(repeat))
    finally:
        _tsa.NUM_SWDGE_GLOBAL_SEMS = _orig_swdge


def _build_program_inner(repeat=1):
    nc = bass.Bass()
    sched = _schedule()
    taps = _taps()

    # xin[k][p=(c,f,hs)][hb'][b][wp]: h = 12*hs + hb, plane rows replicated
    # across the 4 f partition-groups.  kk: +kk cols then -kk cols (ACT
    # bias), col = tap index.
    xin = nc.declare_dram_parameter("xin", [3, 128, HBP, NP, WP], FP16, isOutput=False)
    kkin = nc.declare_dram_parameter("kk", [128, 2 * NTAP], mybir.dt.float32, isOutput=False)
    yout = nc.declare_dram_parameter("yout", [128, HB, NP, W], FP16, isOutput=True)

    with _SplitDrainTC(nc) as tc, ExitStack() as ctx:
        pool = ctx.enter_context(tc.tile_pool(name="main", bufs=1))

        # Compute-instruction ISA slots can encode only ONE sync wait, so
        # "touch" every DMA'd region with a trivial op on each consuming
        # engine right after its DMA; later compute instructions inherit the
        # dependency through engine program order and carry no waits.
        touch_v = pool.tile([1, 64], mybir.dt.float32, name="touch_v", tag="touch_v")
        touch_s = pool.tile([1, 64], mybir.dt.float32, name="touch_s", tag="touch_s")
        touch_g = pool.tile([1, 64], mybir.dt.float32, name="touch_g", tag="touch_g")
        tctr = [0, 0, 0]

        def _touch(src, engines):
            if "v" in engines:
                tctr[0] += 1
                i = tctr[0] % 64
                nc.vector.tensor_scalar_add(touch_v[0:1, i : i + 1], src, 0.0)
            if "s" in engines:
                tctr[1] += 1
                i = tctr[1] % 64
                nc.scalar.copy(touch_s[0:1, i : i + 1], src)
            if "g" in engines:
                tctr[2] += 1
                i = tctr[2] % 64
                nc.gpsimd.tensor_scalar_add(touch_g[0:1, i : i + 1], src, 0.0)

        # which engines read each k-plane tile
        tile_readers = {}
        for ti, (k, dy, dx) in enumerate(taps):
            tile_readers.setdefault(k, set()).add("v" if sched[ti] == "dve" else "s")

        # Plane k=0 and the kk table gate the first tap; planes 1 and 2 are
        # first read 25/50 taps later, so their DMA-dependency "touches" are
        # deferred into the tap loop — compute starts as soon as plane 0
        # lands while the remaining planes stream in behind it.
        in_t = {}
        kkt = pool.tile([128, 2 * NTAP], mybir.dt.float32, name="kkt", tag="kkt")
        # kk is 600B and gates the first sub — land it ahead of the 1.6MB
        # plane transfers on the same queue
        nc.sync.dma_start(kkt[:], kkin[:])
        _touch(kkt[0:1, 0:1], {"v", "s"})
        for k in range(3):
            t = pool.tile([128, HBP, NP, WP], FP16, name=f"in_{k}", tag=f"in_{k}")
            nc.sync.dma_start(t[:], xin[k])
            if k == 0:
                _touch(t[0:1, 0, 0, 0:1], tile_readers[k])
            in_t[k] = t
        touched_planes = {0}

        # Two accumulators, alternated per tap: min(dst, slot, src) with
        # dst != src avoids the SBUF read-modify-write turnaround of an
        # in-place chain (measured ~2.03us vs ~2.2us per [128,4608] min).
        acc = pool.tile([128, HB, NP, W], FP16, name="acc", tag="acc")
        acc2 = pool.tile([128, HB, NP, W], FP16, name="acc2", tag="acc2")
        # NRING large enough that a slot's same-engine WAW producer is >=
        # `lag` own-engine instructions back (only ~2/3 of taps run on ACT,
        # so 14 slots ~ 9+ ACT instructions) and its wait is stripped
        # post-build.
        NRING = 16
        ring = [
            pool.tile([128, HB, NP, W], FP16, name=f"ring_{i}", tag=f"ring_{i}")
            for i in range(NRING)
        ]

        NCHUNK = 4  # final-tap column chunks for early output DMA
        CW = NP * W // NCHUNK  # chunk width in (b,w) cols per hb row: 96
        ring_i = 0
        for _rep in range(repeat):
            for ti, (k, dy, dx) in enumerate(taps):
                eng = sched[ti]
                if k not in touched_planes:
                    _touch(in_t[k][0:1, 0, 0, 0:1], tile_readers[k])
                    touched_planes.add(k)
                win = in_t[k][:, dy : dy + HB, :, dx : dx + W]
                last = ti == len(taps) - 1 and _rep == repeat - 1
                if ti == 0:
                    # first tap initializes acc directly (no min)
                    kk_ap = kkt[:, ti : ti + 1]
                    nc.vector.tensor_scalar(
                        acc[:], win, kk_ap, None, mybir.AluOpType.subtract
                    )
                    cur = acc
                    continue
                if ti == ABS_TI:
                    # absorbed tap: shifted kk is zero here, min the window in
                    dst = acc2 if cur is acc else acc
                    nc.vector.tensor_tensor(
                        dst[:], win, cur[:], mybir.AluOpType.min
                    )
                    cur = dst
                    continue
                if eng == "dve":
                    slot = ring[ring_i % NRING]
                    ring_i += 1
                    kk_ap = kkt[:, ti : ti + 1]
                    nc.vector.tensor_scalar(
                        slot[:], win, kk_ap, None, mybir.AluOpType.subtract
                    )
                else:
                    slot = ring[ring_i % NRING]
                    ring_i += 1
                    negkk_ap = kkt[:, NTAP + ti : NTAP + ti + 1]
                    nc.scalar.activation(
                        slot[:], win,
                        mybir.ActivationFunctionType.Identity, bias=negkk_ap,
                    )
                    # DVE absorber carries the single cross-engine wait on the
                    # ACT sub; the min then inherits it through DVE program
                    # order and keeps its one ISA wait slot free.
                    _touch(slot[0:1, 0, 0, 0:1], {"v"})
                dst = acc2 if cur is acc else acc
                if not last:
                    nc.vector.tensor_tensor(
                        dst[:], slot[:], cur[:], mybir.AluOpType.min
                    )
                    cur = dst
                else:
                    # chunked final min: ship each chunk while the rest
                    # still computes
                    for ci in range(NCHUNK):
                        hb0 = ci * (HB // NCHUNK)
                        hb1 = hb0 + HB // NCHUNK
                        nc.vector.tensor_tensor(
                            dst[:, hb0:hb1], slot[:, hb0:hb1], cur[:, hb0:hb1],
                            mybir.AluOpType.min,
                        )
                        # SP is idle by now; its HWDGE ring generates the
                        # output descriptors in hardware, unlike Pool SWDGE
                        # whose software descriptor-gen adds a multi-us tail
                        nc.sync.dma_start(
                            yout[:, hb0:hb1], dst[:, hb0:hb1]
                        )
                    cur = dst

    return nc


def _get_program(repeat=1):
    key = repeat
    if key not in _prog_cache:
        _prog_cache[key] = _build_program(repeat)
    return _prog_cache[key]


def _krev(kernel):
    """[g, dy, dx, k, c, f] rotated/reversed SE, pure re-indexing of `kernel`."""
    k_ero = np.stack(
        [
            np.rot90(kernel[:, :, 2], k=3, axes=(0, 1)),
            kernel[:, :, 1],
            np.rot90(kernel[:, :, 0], k=1, axes=(0, 1)),
        ],
        axis=2,
    )
    krot = np.stack([np.rot90(k_ero, k=j, axes=(0, 1)) for j in range(4)], axis=0)
    return krot[:, ::-1, ::-1]


def _core_units(core):
    g = core // 2
    fh = core % 2
    return g, list(range(B)), list(range(fh * NF, fh * NF + NF))


def _make_in_map(x, kr, core):
    g, bs, fs = _core_units(core)
    # padded planes ygp[k][b, c, ph, pw] (ph, pw in [0, 100)); partition
    # p = (c, f, hs) reads rows ph = 12*hs + hb', hb' in [0, 16):
    # tile[k][p, hb', b, pw] = ygp[b, c, 12*hs + hb', pw]  (same for all f)
    xin = np.empty((3, 128, HBP, NP, WP), np.float16)
    ridx = 12 * np.arange(HS)[:, None] + np.arange(HBP)[None, :]  # [8, 16] <= 99
    for k in range(3):
        src = x[:, (g + k - 1) % 4]  # [B, H, W, C]
        ygp = np.full((NP, C, H + 2 * PAD, WP), BIG, np.float32)
        for bi, b in enumerate(bs):
            ygp[bi, :, PAD : PAD + H, PAD : PAD + W] = src[b].transpose(2, 0, 1)
        v = ygp[:, :, ridx, :]  # [b, c, hs, hb', pw]
        v = v.transpose(1, 2, 3, 0, 4)  # [c, hs, hb', b, pw]
        # replicate across f: p = 32*c + 8*f + hs
        vv = v.reshape(C, 1, HS, HBP, NP, WP)
        vv = np.broadcast_to(vv, (C, NF, HS, HBP, NP, WP))
        xin[k] = vv.reshape(128, HBP, NP, WP).astype(np.float16)
    # kk columns: +kk then -kk, col = tap index in _taps() order,
    # row p = (c, f, hs): value kr[g, dy, dx, k, c, fs[f]], shifted so the
    # absorbed tap's entry is zero (host corrects the sum in _assemble)
    sel = kr[g][:, :, :, :, fs]  # [dy, dx, k, c, NF]
    tap_cf = sel.transpose(2, 0, 1, 3, 4).reshape(NTAP, C, NF)  # canonical [ti, c, f]
    order = [k * KH * KW + dy * KW + dx for (k, dy, dx) in _taps()]
    tap_cf = tap_cf[order]
    _CORR[core] = tap_cf[ABS_TI].sum(axis=0)  # [NF], channel-summed offset
    tap_cf = tap_cf - tap_cf[ABS_TI]
    kk = np.empty((128, 2 * NTAP), np.float32)
    col = tap_cf.transpose(1, 2, 0)  # [c, f, ti]
    col = np.repeat(col.reshape(C, NF, 1, NTAP), HS, axis=2).reshape(128, NTAP)
    kk[:, :NTAP] = col
    kk[:, NTAP:] = -col
    return {"xin": xin, "kk": kk}


def _assemble(results):
    out = np.zeros((B, G, H, W, F), np.float32)
    for core in range(N_CORES):
        g, bs, fs = _core_units(core)
        y = np.asarray(results[core]["yout"]).astype(np.float32)
        # y[p=(c,f,hs), hb, b, w] -> sum over c -> out[b, g, 12*hs+hb, w, f]
        y = y.reshape(C, NF, HS, HB, NP, W).sum(axis=0)  # [f, hs, hb, b, w]
        y = y - _CORR[core][:, None, None, None, None]
        y = y.transpose(3, 1, 2, 4, 0)  # [b, hs, hb, w, f]
        y = y.reshape(NP, H, W, NF)
        for bi, b in enumerate(bs):
            out[b, g, :, :, fs[0] : fs[0] + NF] = y[bi]
    return out


def kernel(x, kernel):
    x = np.ascontiguousarray(np.asarray(x, dtype=np.float32))
    se = np.ascontiguousarray(np.asarray(kernel, dtype=np.float32))
    kr = _krev(se)  # [g, dy, dx, k, c, f]
    in_maps = [_make_in_map(x, kr, core) for core in range(N_CORES)]
    nc = _get_program(CFG_REPEAT)
    res = run_bass_kernel_spmd(nc, in_maps, list(range(N_CORES)), trace=False)
    global LAST_RESULTS
    LAST_RESULTS = res
    return _assemble(res.results)


# revision 30
# speedup vs baseline: 1.0053x; 1.0053x over previous
"""Trainium2 Bass kernel for ErosionP4 (P4 group-equivariant grayscale erosion).

Reference computation (shapes hardcoded):
  x: [B=4, G=4, H=96, W=96, C=4] fp32, kernel: [5, 5, 3, C=4, F=8] fp32
  out[b,g,h,w,f] = sum_c min_{k,dy,dx} ( ygp[b,g,k,h+dy,w+dx,c] - krev[g,dy,dx,k,c,f] )
  where ygp[b,g,k] = x[b, (g+k-1) mod 4] spatially padded with +inf and
  krev = the 4 planar rotations of the depth-rotated SE, spatially reversed.

Sharding: core -> (g = core//2, f-half = core%2).  Each core computes all 4
batches for one group-rotation g and 4 of the 8 filters.

Layout "cf-block": partition p = 32*c + 8*f + hs (c = channel, f = filter
within the half, hs = h//12), free dims (hb, b, w) with h = 12*hs + hb.
Every (c, f) pair owns its own partitions, so the per-tap SE value is a
per-partition scalar and ONE tensor_scalar sub + ONE tensor_tensor min of
[128, 4608] cover all four filters of a tap:

  per tap:  tmp = win - kk[(c,f)]   (ACT activation w/ bias, 1 elem/cy,
                                     or DVE tensor_scalar, 4 elem/cy)
            acc = min(acc, tmp)     (DVE tensor_tensor, 2 elem/cy)

The input plane data is replicated across the 4 f partition groups (host-side
prep; DMA is a few us, off the critical path).  The 75 taps are split between
ACT-sub and DVE-sub by a static greedy balance with HW-measured costs
(DVE min 2150ns, DVE sub 1075ns, ACT sub 3900ns per tap): DVE streams every
min (the 2-elem/cycle two-tensor-op wall) while ACT shoulders most subs.
This sits at the measured two-engine roofline (~190us steady-state) with
~6x fewer instructions and semaphores than a per-filter formulation, which
cuts the one-shot (startup + drain) cost of a single NEFF execution.

The channel sum happens on the host (c pieces live on different partitions).
The final tap is processed in 4 column chunks so each chunk's Pool-sequencer
SWDGE output DMA starts while the rest still computes.
"""

import os
from contextlib import ExitStack

import numpy as np

import concourse.bass as bass
import concourse.mybir as mybir
import concourse.tile as tile
from concourse.bass_utils import run_bass_kernel_spmd

B, G, H, W, C = 4, 4, 96, 96, 4
KH, KW, F = 5, 5, 8
PAD = 2
WP = W + PAD * 2  # 100
NTAP = 3 * KH * KW  # 75
N_CORES = 8
NP = 4  # batches per core
NF = F // 2  # filters per core
HS = 8  # h slices (partition sub-dim)
HB = 12  # h rows per slice (free dim)
HBP = HB + KH - 1  # 16 padded rows per slice
FREE = HB * NP * W  # 4608
BIG = 30000.0  # +inf stand-in that survives fp16

CFG_REPEAT = int(os.environ.get("KCFG_REPEAT", "1"))
# HW-measured per-tap costs (ns) for the static DVE/ACT schedule.
COST_DVE_SUB = float(os.environ.get("KCFG_DVE_SUB", "1100"))
COST_DVE_MIN = float(os.environ.get("KCFG_DVE_MIN", "2170"))
COST_ACT_SUB = float(os.environ.get("KCFG_ACT_COST", "3400"))

FP16 = mybir.dt.float16

_prog_cache = {}
LAST_RESULTS = None
_CORR = {}  # core -> [NF] channel-summed absorbed-tap offset (set by _make_in_map)


def _taps():
    """Tap order is free (min is commutative).  Entry 1 is swapped to an
    even-dx tap: its SE value is absorbed into the accumulator offset (the
    whole kk table is shifted by it and the host subtracts the channel-summed
    offset), so its min reads the input window directly — no sub and no
    ACT round-trip at the pipeline head.  Even dx keeps the window 4B-aligned
    for the min's 2x packed mode."""
    t = [(k, dy, dx) for k in range(3) for dy in range(KH) for dx in range(KW)]
    t[1], t[2] = t[2], t[1]  # t[1] = (0, 0, 2)
    return t


ABS_TI = 1  # index (in _taps() order) of the absorbed tap


def _schedule():
    """Greedy static assignment of the 75 tap-subs to 'dve' | 'act'.

    Simulates both engine timelines: a DVE-own tap costs DVE sub+min; an
    ACT-assist tap costs ACT the sub, and DVE pays the min once the sub has
    landed (cross-engine wait)."""
    t_dve = t_act = 0.0
    out = []
    for ti in range(NTAP):
        if ti == 0:
            out.append("dve")
            t_dve += COST_DVE_SUB  # init writes acc directly, no min
            continue
        end_dve = t_dve + COST_DVE_SUB + COST_DVE_MIN
        a_end = t_act + COST_ACT_SUB
        end_act = max(t_dve, a_end) + COST_DVE_MIN
        if end_dve <= end_act:
            out.append("dve")
            t_dve = end_dve
        else:
            out.append("act")
            t_act = a_end
            t_dve = end_act
    # the final tap feeds the chunked output mins; keep it off ACT so the
    # output DMAs never queue behind ACT's sub backlog
    out[-1] = "dve"
    # the greedy bifurcates between ~54 and ~72 ACT taps; ACT_EXTRA flips a
    # few more DVE taps to ACT (spread evenly) for finer balance control
    extra = int(os.environ.get("KCFG_ACT_EXTRA", "0"))
    if extra > 0:
        dve_idx = [i for i, e in enumerate(out) if e == "dve" and 0 < i < NTAP - 1]
        step = max(1, len(dve_idx) // extra)
        for j in dve_idx[::step][:extra]:
            out[j] = "act"
    return out


class _SplitDrainTC(tile.TileContext):
    """TileContext whose kernel-tail drain is split into one drain per sem
    lane: the stock single Drain carries a wait for every lane used, which
    overflows the CTRL struct's sync-wait encoding on this compiler."""

    def _drain_and_barrier(self, tick_clock, wait_clock):
        from concourse.tile_sem_assignment import N_PROCS
        from concourse.vector_clock import ScopedClock, VectorClock

        gc = tick_clock.global_clock
        ticks = [gc[p] for p in range(N_PROCS)]
        for p in range(N_PROCS):
            if ticks[p] <= 0:
                continue
            sub = [ticks[q] if q == p else 0 for q in range(N_PROCS)]
            d = self.nc.sync.drain()
            wait_clock.add_sem_waits(d.ins, ScopedClock({None: VectorClock(sub)}))

        self.nc.all_engine_barrier()
        assert self.sems is not None
        popped = self.nc._tile_sem_poison_stack.pop()
        assert popped is self._sem_poison
        self.nc.clear_and_free_semaphores(list(self.sems.allocated().values()))
        self.nc.all_engine_barrier()


def _strip_stale_same_engine_waits(nc, lag=7):
    """Drop same-engine sem waits whose producer finished >= `lag` own-engine
    instructions earlier.

    This tile version emits a sem wait for EVERY hazard, including same-engine
    WAW/WAR whose producers are long retired; compute ISA structs can encode
    only ONE sync wait, so a ring-buffer rewrite (same-engine WAW + cross-
    engine WAR) overflows codegen.  Engines issue in order and their writes
    land within a couple of instructions, so a same-engine wait on a producer
    `lag` instructions back is vacuous.  Recent same-engine waits (pipelined
    RAW guards) are kept.
    """
    strip_types = {
        "InstActivation", "InstTensorScalarPtr", "InstTensorTensor",
        "InstTensorScalar", "InstMemset", "InstCopy", "InstTensorCopy",
        "InstTensorReduce",
    }
    counts = {}
    fn = nc.m.functions[0]
    for blk in fn.blocks:
        for ins in blk.instructions:
            si = ins.sync_info
            if si is None:
                continue
            eng = getattr(ins, "engine", None)
            ename = getattr(eng, "name", None) or (str(eng).split(".")[-1] if eng else "")
            if si.on_wait and type(ins).__name__ in strip_types and ename in (
                "Activation", "DVE", "Pool", "PE"
            ):
                keep = []
                for w in si.on_wait:
                    nm = w.ant_name or ""
                    if (
                        nm.startswith(ename + "_")
                        and w.wait_mode == "sem-ge-imm"
                        and w.wait_value is not None
                        and counts.get(nm, 0) - w.wait_value >= lag
                    ):
                        continue
                    keep.append(w)
                if len(keep) != len(si.on_wait):
                    si.on_wait = keep
            for u in si.on_update or []:
                if u.ant_name:
                    counts[u.ant_name] = counts.get(u.ant_name, 0) + (u.update_value or 1)
    return nc


def _build_program(repeat=1):
    import concourse.tile_sem_assignment as _tsa

    _orig_swdge = _tsa.NUM_SWDGE_GLOBAL_SEMS
    _tsa.NUM_SWDGE_GLOBAL_SEMS = 4
    try:
        return _strip_stale_same_engine_waits(_build_program_inner(repeat))
    finally:
        _tsa.NUM_SWDGE_GLOBAL_SEMS = _orig_swdge


def _build_program_inner(repeat=1):
    nc = bass.Bass()
    sched = _schedule()
    taps = _taps()

    # xin[k][p=(c,f,hs)][hb'][b][wp]: h = 12*hs + hb, plane rows replicated
    # across the 4 f partition-groups.  kk: +kk cols then -kk cols (ACT
    # bias), col = tap index.
    xin = nc.declare_dram_parameter("xin", [3, 128, HBP, NP, WP], FP16, isOutput=False)
    kkin = nc.declare_dram_parameter("kk", [128, 2 * NTAP], mybir.dt.float32, isOutput=False)
    yout = nc.declare_dram_parameter("yout", [128, HB, NP, W], FP16, isOutput=True)

    with _SplitDrainTC(nc) as tc, ExitStack() as ctx:
        pool = ctx.enter_context(tc.tile_pool(name="main", bufs=1))

        # Compute-instruction ISA slots can encode only ONE sync wait, so
        # "touch" every DMA'd region with a trivial op on each consuming
        # engine right after its DMA; later compute instructions inherit the
        # dependency through engine program order and carry no waits.
        touch_v = pool.tile([1, 64], mybir.dt.float32, name="touch_v", tag="touch_v")
        touch_s = pool.tile([1, 64], mybir.dt.float32, name="touch_s", tag="touch_s")
        touch_g = pool.tile([1, 64], mybir.dt.float32, name="touch_g", tag="touch_g")
        tctr = [0, 0, 0]

        def _touch(src, engines):
            if "v" in engines:
                tctr[0] += 1
                i = tctr[0] % 64
                nc.vector.tensor_scalar_add(touch_v[0:1, i : i + 1], src, 0.0)
            if "s" in engines:
                tctr[1] += 1
                i = tctr[1] % 64
                nc.scalar.copy(touch_s[0:1, i : i + 1], src)
            if "g" in engines:
                tctr[2] += 1
                i = tctr[2] % 64
                nc.gpsimd.tensor_scalar_add(touch_g[0:1, i : i + 1], src, 0.0)

        # which engines read each k-plane tile
        tile_readers = {}
        for ti, (k, dy, dx) in enumerate(taps):
            tile_readers.setdefault(k, set()).add("v" if sched[ti] == "dve" else "s")

        # Plane k=0 and the kk table gate the first tap; planes 1 and 2 are
        # first read 25/50 taps later, so their DMA-dependency "touches" are
        # deferred into the tap loop — compute starts as soon as plane 0
        # lands while the remaining planes stream in behind it.
        in_t = {}
        kkt = pool.tile([128, 2 * NTAP], mybir.dt.float32, name="kkt", tag="kkt")
        # kk is 600B and gates the first sub — land it ahead of the 1.6MB
        # plane transfers on the same queue
        nc.sync.dma_start(kkt[:], kkin[:])
        _touch(kkt[0:1, 0:1], {"v", "s"})
        for k in range(3):
            t = pool.tile([128, HBP, NP, WP], FP16, name=f"in_{k}", tag=f"in_{k}")
            nc.sync.dma_start(t[:], xin[k])
            if k == 0:
                _touch(t[0:1, 0, 0, 0:1], tile_readers[k])
            in_t[k] = t
        touched_planes = {0}

        # Three rotating accumulators: min(dst, slot, src) with dst != src
        # avoids the SBUF read-modify-write turnaround of an in-place chain,
        # and the 3-deep rotation also avoids writing the tile the PREVIOUS
        # min just read (1-back WAR) — measured ~2.14us vs ~2.29us (2-buf)
        # vs ~2.2-2.3us (in-place) per [128,4608] min.
        acc = pool.tile([128, HB, NP, W], FP16, name="acc", tag="acc")
        acc2 = pool.tile([128, HB, NP, W], FP16, name="acc2", tag="acc2")
        acc3 = pool.tile([128, HB, NP, W], FP16, name="acc3", tag="acc3")
        nxt_acc = {id(acc): acc2, id(acc2): acc3, id(acc3): acc}
        # NRING large enough that a slot's same-engine WAW producer is >=
        # `lag` own-engine instructions back (only ~2/3 of taps run on ACT,
        # so 14 slots ~ 9+ ACT instructions) and its wait is stripped
        # post-build.
        NRING = 15
        ring = [
            pool.tile([128, HB, NP, W], FP16, name=f"ring_{i}", tag=f"ring_{i}")
            for i in range(NRING)
        ]

        NCHUNK = 4  # final-tap column chunks for early output DMA
        CW = NP * W // NCHUNK  # chunk width in (b,w) cols per hb row: 96
        ring_i = 0
        for _rep in range(repeat):
            for ti, (k, dy, dx) in enumerate(taps):
                eng = sched[ti]
                if k not in touched_planes:
                    _touch(in_t[k][0:1, 0, 0, 0:1], tile_readers[k])
                    touched_planes.add(k)
                win = in_t[k][:, dy : dy + HB, :, dx : dx + W]
                last = ti == len(taps) - 1 and _rep == repeat - 1
                if ti == 0:
                    # first tap initializes acc directly (no min)
                    kk_ap = kkt[:, ti : ti + 1]
                    nc.vector.tensor_scalar(
                        acc[:], win, kk_ap, None, mybir.AluOpType.subtract
                    )
                    cur = acc
                    continue
                if ti == ABS_TI:
                    # absorbed tap: shifted kk is zero here, min the window in
                    dst = nxt_acc[id(cur)]
                    nc.vector.tensor_tensor(
                        dst[:], win, cur[:], mybir.AluOpType.min
                    )
                    cur = dst
                    continue
                if eng == "dve":
                    slot = ring[ring_i % NRING]
                    ring_i += 1
                    kk_ap = kkt[:, ti : ti + 1]
                    nc.vector.tensor_scalar(
                        slot[:], win, kk_ap, None, mybir.AluOpType.subtract
                    )
                else:
                    slot = ring[ring_i % NRING]
                    ring_i += 1
                    negkk_ap = kkt[:, NTAP + ti : NTAP + ti + 1]
                    nc.scalar.activation(
                        slot[:], win,
                        mybir.ActivationFunctionType.Identity, bias=negkk_ap,
                    )
                    # DVE absorber carries the single cross-engine wait on the
                    # ACT sub; the min then inherits it through DVE program
                    # order and keeps its one ISA wait slot free.
                    _touch(slot[0:1, 0, 0, 0:1], {"v"})
                dst = nxt_acc[id(cur)]
                if not last:
                    nc.vector.tensor_tensor(
                        dst[:], slot[:], cur[:], mybir.AluOpType.min
                    )
                    cur = dst
                else:
                    # chunked final min: ship each chunk while the rest
                    # still computes
                    for ci in range(NCHUNK):
                        hb0 = ci * (HB // NCHUNK)
                        hb1 = hb0 + HB // NCHUNK
                        nc.vector.tensor_tensor(
                            dst[:, hb0:hb1], slot[:, hb0:hb1], cur[:, hb0:hb1],
                            mybir.AluOpType.min,
                        )
                        # SP is idle by now; its HWDGE ring generates the
                        # output descriptors in hardware, unlike Pool SWDGE
                        # whose software descriptor-gen adds a multi-us tail
                        nc.sync.dma_start(
                            yout[:, hb0:hb1], dst[:, hb0:hb1]
                        )
                    cur = dst

    return nc


def _get_program(repeat=1):
    key = repeat
    if key not in _prog_cache:
        _prog_cache[key] = _build_program(repeat)
    return _prog_cache[key]


def _krev(kernel):
    """[g, dy, dx, k, c, f] rotated/reversed SE, pure re-indexing of `kernel`."""
    k_ero = np.stack(
        [
            np.rot90(kernel[:, :, 2], k=3, axes=(0, 1)),
            kernel[:, :, 1],
            np.rot90(kernel[:, :, 0], k=1, axes=(0, 1)),
        ],
        axis=2,
    )
    krot = np.stack([np.rot90(k_ero, k=j, axes=(0, 1)) for j in range(4)], axis=0)
    return krot[:, ::-1, ::-1]


def _core_units(core):
    g = core // 2
    fh = core % 2
    return g, list(range(B)), list(range(fh * NF, fh * NF + NF))


def _make_in_map(x, kr, core):
    g, bs, fs = _core_units(core)
    # padded planes ygp[k][b, c, ph, pw] (ph, pw in [0, 100)); partition
    # p = (c, f, hs) reads rows ph = 12*hs + hb', hb' in [0, 16):
    # tile[k][p, hb', b, pw] = ygp[b, c, 12*hs + hb', pw]  (same for all f)
    xin = np.empty((3, 128, HBP, NP, WP), np.float16)
    ridx = 12 * np.arange(HS)[:, None] + np.arange(HBP)[None, :]  # [8, 16] <= 99
    for k in range(3):
        src = x[:, (g + k - 1) % 4]  # [B, H, W, C]
        ygp = np.full((NP, C, H + 2 * PAD, WP), BIG, np.float32)
        for bi, b in enumerate(bs):
            ygp[bi, :, PAD : PAD + H, PAD : PAD + W] = src[b].transpose(2, 0, 1)
        v = ygp[:, :, ridx, :]  # [b, c, hs, hb', pw]
        v = v.transpose(1, 2, 3, 0, 4)  # [c, hs, hb', b, pw]
        # replicate across f: p = 32*c + 8*f + hs
        vv = v.reshape(C, 1, HS, HBP, NP, WP)
        vv = np.broadcast_to(vv, (C, NF, HS, HBP, NP, WP))
        xin[k] = vv.reshape(128, HBP, NP, WP).astype(np.float16)
    # kk columns: +kk then -kk, col = tap index in _taps() order,
    # row p = (c, f, hs): value kr[g, dy, dx, k, c, fs[f]], shifted so the
    # absorbed tap's entry is zero (host corrects the sum in _assemble)
    sel = kr[g][:, :, :, :, fs]  # [dy, dx, k, c, NF]
    tap_cf = sel.transpose(2, 0, 1, 3, 4).reshape(NTAP, C, NF)  # canonical [ti, c, f]
    order = [k * KH * KW + dy * KW + dx for (k, dy, dx) in _taps()]
    tap_cf = tap_cf[order]
    _CORR[core] = tap_cf[ABS_TI].sum(axis=0)  # [NF], channel-summed offset
    tap_cf = tap_cf - tap_cf[ABS_TI]
    kk = np.empty((128, 2 * NTAP), np.float32)
    col = tap_cf.transpose(1, 2, 0)  # [c, f, ti]
    col = np.repeat(col.reshape(C, NF, 1, NTAP), HS, axis=2).reshape(128, NTAP)
    kk[:, :NTAP] = col
    kk[:, NTAP:] = -col
    return {"xin": xin, "kk": kk}


def _assemble(results):
    out = np.zeros((B, G, H, W, F), np.float32)
    for core in range(N_CORES):
        g, bs, fs = _core_units(core)
        y = np.asarray(results[core]["yout"]).astype(np.float32)
        # y[p=(c,f,hs), hb, b, w] -> sum over c -> out[b, g, 12*hs+hb, w, f]
        y = y.reshape(C, NF, HS, HB, NP, W).sum(axis=0)  # [f, hs, hb, b, w]
        y = y - _CORR[core][:, None, None, None, None]
        y = y.transpose(3, 1, 2, 4, 0)  # [b, hs, hb, w, f]
        y = y.reshape(NP, H, W, NF)
        for bi, b in enumerate(bs):
            out[b, g, :, :, fs[0] : fs[0] + NF] = y[bi]
    return out


def kernel(x, kernel):
    x = np.ascontiguousarray(np.asarray(x, dtype=np.float32))
    se = np.ascontiguousarray(np.asarray(kernel, dtype=np.float32))
    kr = _krev(se)  # [g, dy, dx, k, c, f]
    in_maps = [_make_in_map(x, kr, core) for core in range(N_CORES)]
    nc = _get_program(CFG_REPEAT)
    res = run_bass_kernel_spmd(nc, in_maps, list(range(N_CORES)), trace=False)
    global LAST_RESULTS
    LAST_RESULTS = res
    return _assemble(res.results)
